# revision 15
# baseline (speedup 1.0000x reference)
"""Trainium2 Bass kernel for nn_MultiHeadAttention (B=2, S=2048, H=16, d_model=1024).

Sharding (8 cores): data-parallel over batch (2) x tensor-parallel over heads
(4 heads per core, Megatron-style column/row split of the Q/K/V/O projections).
Each core computes a partial output [S, d_model] for its batch; the host sums
the 4 partials per batch and adds the output bias.

v2: bf16 operands end-to-end (halves HBM traffic + LDWEIGHTS time), row-tiled
K=64 score matmuls (two heads concurrent on disjoint PE row groups), softmax
exp split across ScalarE (exact) / VectorE / GpSimd (Schraudolph int-trick,
single fused tensor_scalar each), causal diagonal masking via a precomputed
mask multiply on DVE, and softmax denominators taken as reciprocal straight
from PSUM then partition-broadcast via SBUF->SBUF DMA.
"""
import sys

for _p in ("/opt/trn_rl_repo", "/root/.axon_site/_ro/trn_rl_repo"):
    if _p not in sys.path:
        sys.path.insert(0, _p)

import math

import numpy as np
from ml_dtypes import bfloat16

import concourse.bass as bass  # noqa: F401
import concourse.mybir as mybir
from concourse import bacc
from concourse.tile import TileContext
from concourse.tile import add_dep_helper
from concourse.bass_utils import run_bass_kernel_spmd

H = 16
D_MODEL = 1024
D_K = 64
B, S = 2, 2048
N_CORES = 8
HEADS_PER_CORE = 4
E = HEADS_PER_CORE * D_K  # 256 output channels per core
CH = 512                  # tq chunk width
N_CH = S // CH            # 4 chunks
N_TB = S // 128           # 16 token blocks

F32 = mybir.dt.float32
F32R = mybir.dt.float32r
BF16 = mybir.dt.bfloat16
I16 = mybir.dt.int16
EXP = mybir.ActivationFunctionType.Exp
MULT = mybir.AluOpType.mult
ADD = mybir.AluOpType.add

# Schraudolph exp (bf16): exp(0.125*x) ~= bitcast_bf16(int16(SCHR_A*x + SCHR_B))
SCHR_A = 0.125 * (1 << 7) / math.log(2.0)
SCHR_B = (127.0 - 0.0437) * (1 << 7)

ROW_TILED = True          # K=64 row-tiled score matmuls (else zero-padded K=128)
# exp engine rotation within each 4-tile group: S=scalar exact, V/G=Schraudolph
EXP_ROT = ("S", "V", "S", "V")
DIAG_ROT = ("S", "V", "V", "V")

_NC_CACHE = None


def build_nc():
    nc = bacc.Bacc("TRN2", target_bir_lowering=False, debug=False,
                   enable_asserts=False)
    # x tensors host-packed as [p, chunk, kd, t] so each chunk DMA is 128
    # contiguous rows
    xq = nc.dram_tensor("xq", (128, N_CH, 8, CH), BF16, kind="ExternalInput").ap()
    xk = nc.dram_tensor("xk", (128, N_CH, 8, CH), BF16, kind="ExternalInput").ap()
    xv = nc.dram_tensor("xv", (128, N_CH, 8, CH), BF16, kind="ExternalInput").ap()
    wq = nc.dram_tensor("wq", (128, 8, E), BF16, kind="ExternalInput").ap()
    wk = nc.dram_tensor("wk", (128, 8, E), BF16, kind="ExternalInput").ap()
    wv = nc.dram_tensor("wv", (128, 8, E), BF16, kind="ExternalInput").ap()
    wo = nc.dram_tensor("wo", (128, 2, D_MODEL), BF16, kind="ExternalInput").ap()
    bq = nc.dram_tensor("bq", (128, 2), F32, kind="ExternalInput").ap()
    bk = nc.dram_tensor("bk", (128, 2), F32, kind="ExternalInput").ap()
    part = nc.dram_tensor("part", (S, D_MODEL), BF16, kind="ExternalOutput").ap()

    with TileContext(nc) as tc:
        with tc.tile_pool(name="const", bufs=1) as cp, \
             tc.tile_pool(name="xc", bufs=5) as xcp, \
             tc.tile_pool(name="wk_", bufs=3) as wkp, \
             tc.tile_pool(name="pp", bufs=2, space="PSUM") as ppp, \
             tc.tile_pool(name="etp", bufs=2, space="PSUM") as etpp, \
             tc.tile_pool(name="ctxp", bufs=1, space="PSUM") as ctxp:

            # ---- one-time loads; tiny/bias/mask work first so the gpsimd
            # library reload happens during the DMA head ------------------
            bq_sb = cp.tile([128, 2], F32, tag="bq_sb")
            bk_sb = cp.tile([128, 2], F32, tag="bk_sb")
            nc.scalar.dma_start(bq_sb[:], bq[:])
            nc.scalar.dma_start(bk_sb[:], bk[:])

            # causal masks for the 4 diagonal tile rows: keep t >= 128*r + p
            dmask = cp.tile([128, 4, CH], BF16, tag="dmask")
            nc.vector.memset(dmask[:], 1.0)
            # Bmask: SCHR_B where causal, -1e9 where masked -> int16 output
            # saturates to -32768 = bf16 -0.0 (an exact zero probability)
            bmask = cp.tile([128, 4, CH], F32, tag="bmask")
            nc.vector.memset(bmask[:], SCHR_B)
            for r in range(4):
                nc.gpsimd.affine_select(
                    out=dmask[:, r, :], in_=dmask[:, r, :],
                    pattern=[[1, CH]], base=-r * 128,
                    channel_multiplier=-1,
                    compare_op=mybir.AluOpType.is_ge,
                    fill=0.0)
                nc.gpsimd.affine_select(
                    out=bmask[:, r, :], in_=bmask[:, r, :],
                    pattern=[[1, CH]], base=-r * 128,
                    channel_multiplier=-1,
                    compare_op=mybir.AluOpType.is_ge,
                    fill=-1e9)

            wq_sb = cp.tile([128, 8, E], BF16, tag="wq_sb")
            wk_sb = cp.tile([128, 8, E], BF16, tag="wk_sb")
            wv_sb = cp.tile([128, 8, E], BF16, tag="wv_sb")
            wo_sb = cp.tile([128, 2, D_MODEL], BF16, tag="wo_sb")
            nc.scalar.dma_start(wq_sb[:], wq[:])

            # persistent activations (bf16)
            if ROW_TILED:
                # qT[p]: [128, S], partitions 0-63 head A / 64-127 head B
                qT = [cp.tile([128, S], BF16, tag=f"qT{p}", name=f"qT{p}")
                      for p in range(2)]
            else:
                # zero-padded per head ([qA;0] / [0;qB]) so score matmuls run
                # K=128 against the full k pair tile
                qTz = [[cp.tile([128, S], BF16, tag=f"qTz{p}{h}",
                                name=f"qTz{p}{h}") for h in range(2)]
                       for p in range(2)]
                for p in range(2):
                    nc.vector.memset(qTz[p][0][64:128, :], 0.0)
                    nc.vector.memset(qTz[p][1][0:64, :], 0.0)
            kT = [cp.tile([128, S], BF16, tag=f"kT{p}", name=f"kT{p}")
                  for p in range(2)]
            # va[p]: [t, 130] per tb: [vA(64) | onesA | vB(64) | onesB]
            va = [cp.tile([128, N_TB, 130], BF16, tag=f"va{p}", name=f"va{p}")
                  for p in range(2)]
            ctxT = [cp.tile([128, S], BF16, tag=f"ctxT{p}", name=f"ctxT{p}")
                    for p in range(2)]
            for p in range(2):
                nc.vector.memset(va[p][:, :, 64:65], 1.0)
                nc.vector.memset(va[p][:, :, 129:130], 1.0)

            # ---- main chunk loop ------------------------------------------
            def load_xc(src, c, gate, fine=False):
                if fine:
                    # chunk 0: four separate quarter tiles, one DMA each, so
                    # per-tile dep tracking lets the first matmuls start as
                    # soon as their quarter lands
                    quarters = []
                    for qi in range(4):
                        xq_ = xcp.tile([128, 2, CH], BF16, tag="xcq",
                                       name="xcq", bufs=6)
                        d = nc.sync.dma_start(
                            xq_[:], src[:, c, 2 * qi:2 * qi + 2, :])
                        if gate is not None:
                            add_dep_helper(d.ins, gate.ins, sync=True,
                                           reason="dma-throttle")
                        quarters.append(xq_)
                    return lambda kd: quarters[kd // 2][:, kd % 2, :]
                # two half tiles for finer prefetch rotation
                halves = []
                for half in range(2):
                    xh = xcp.tile([128, 4, CH], BF16, tag="xc", name="xc")
                    d = nc.sync.dma_start(
                        xh[:], src[:, c, 4 * half:4 * half + 4, :])
                    if gate is not None:
                        add_dep_helper(d.ins, gate.ins, sync=True,
                                       reason="dma-throttle")
                    halves.append(xh)
                return lambda kd: halves[kd // 4][:, kd % 4, :]

            def emit_proj(c):
                csl = slice(c * CH, (c + 1) * CH)
                gates = {}
                # q/k projections -> qT/kT[e, t-chunk]; on chunk 0, stagger
                # the k and v loads behind the previous tensor's first MMs
                # so the first matmul inputs aren't stuck behind the DMA head
                for name_ in ("q", "k"):
                    src = xq if name_ == "q" else xk
                    w_sb = wq_sb if name_ == "q" else wk_sb
                    b_sb = bq_sb if name_ == "q" else bk_sb
                    gate = None
                    if c == 0:
                        gate = gates.get("q" if name_ == "k" else None)
                    xcs = load_xc(src, c, gate, fine=(c == 0))
                    if name_ == "k" and c == 0:
                        nc.sync.dma_start(wk_sb[:], wk[:])
                    for eb in range(2):
                        pps = ppp.tile([128, CH], F32, tag="pp", name="pp")
                        for kd in range(8):
                            mm = nc.tensor.matmul(
                                pps[:],
                                w_sb[:, kd, eb * 128:(eb + 1) * 128],
                                xcs(kd),
                                start=(kd == 0), stop=(kd == 7))
                            if eb == 0 and kd == 0:
                                gates[name_] = mm
                        if ROW_TILED:
                            dst = qT[eb] if name_ == "q" else kT[eb]
                            nc.vector.tensor_scalar_add(
                                dst[:, csl], pps[:], b_sb[:, eb:eb + 1])
                        else:
                            if name_ == "q":
                                nc.vector.tensor_scalar_add(
                                    qTz[eb][0][0:64, csl], pps[0:64, :],
                                    b_sb[0:64, eb:eb + 1])
                                nc.vector.tensor_scalar_add(
                                    qTz[eb][1][64:128, csl], pps[64:128, :],
                                    b_sb[64:128, eb:eb + 1])
                            else:
                                nc.vector.tensor_scalar_add(
                                    kT[eb][:, csl], pps[:], b_sb[:, eb:eb + 1])

                # v projection -> va[t, e] with ones columns at 0/129
                if c == 0:
                    d = nc.sync.dma_start(wv_sb[:], wv[:])
                    add_dep_helper(d.ins, gates["q"].ins, sync=True,
                                   reason="dma-throttle")
                xcs = load_xc(xv, c, gates.get("k") if c == 0 else None,
                              fine=(c == 0))
                for j in range(4):
                    tb = 4 * c + j
                    vps = ppp.tile([128, E], F32, tag="pp", name="pp")
                    for kd in range(8):
                        nc.tensor.matmul(
                            vps[:],
                            xcs(kd)[:, j * 128:(j + 1) * 128],
                            wv_sb[:, kd, :],
                            start=(kd == 0), stop=(kd == 7))
                    for p in range(2):
                        dst = va[p][:, tb, :].rearrange(
                            "q (g x) -> q g x", g=2, x=65)[:, :, 0:64]
                        nc.scalar.copy(
                            dst,
                            vps[:, 128 * p:128 * p + 128].rearrange(
                                "q (g x) -> q g x", g=2, x=64))

            def emit_attn(c):
                csl = slice(c * CH, (c + 1) * CH)
                n_tkb = 4 * (c + 1)
                for p in range(2):
                    cps = [ctxp.tile([65, CH], F32, tag=f"ctx{hh}",
                                     name=f"ctx{hh}") for hh in range(2)]
                    for g in range(c + 1):
                        ets_group = {}
                        for i, tkb in enumerate(range(4 * g, 4 * g + 4)):
                            # diag tiles: columns 0:128r are fully above the
                            # causal boundary -- skip them in the score MM,
                            # the exp, and the ctx MM
                            r = tkb - 4 * c
                            cl0 = 128 * r if g == c else 0
                            nn = CH - cl0
                            qsl = slice(c * CH + cl0, (c + 1) * CH)
                            etps = etpp.tile([128, 2, CH], F32, tag="et",
                                             name="et")
                            for hh in range(2):
                                if ROW_TILED:
                                    nc.tensor.matmul(
                                        etps[:, hh, cl0:],
                                        kT[p][64 * hh:64 * hh + 64,
                                              tkb * 128:(tkb + 1) * 128],
                                        qT[p][64 * hh:64 * hh + 64, qsl],
                                        start=True, stop=True)
                                else:
                                    nc.tensor.matmul(
                                        etps[:, hh, cl0:],
                                        kT[p][:, tkb * 128:(tkb + 1) * 128],
                                        qTz[p][hh][:, qsl],
                                        start=True, stop=True)
                            ets = wkp.tile([128, 2, CH], BF16, tag="ets",
                                           name="ets", bufs=4)
                            eng = DIAG_ROT[i] if g == c else EXP_ROT[i]
                            if eng == "S":
                                nc.scalar.activation(ets[:, :, cl0:],
                                                     etps[:, :, cl0:], EXP,
                                                     scale=0.125)
                                if g == c and cl0 < 384:
                                    # diagonal block: causal mask multiply
                                    mk = dmask[:, r, cl0:].unsqueeze(1)
                                    mk = mk.broadcast_to([128, 2, nn])
                                    nc.vector.tensor_mul(
                                        ets[:, :, cl0:], ets[:, :, cl0:], mk)
                            elif g == c:
                                # fused Schraudolph exp + causal mask: B is
                                # -1e9 on masked lanes -> int16 saturates to
                                # -32768 = bf16 -0.0 (an exact zero)
                                bm = bmask[:, r, cl0:].unsqueeze(1)
                                bm = bm.broadcast_to([128, 2, nn])
                                nc.vector.scalar_tensor_tensor(
                                    ets[:, :, cl0:].bitcast(I16),
                                    etps[:, :, cl0:],
                                    SCHR_A, bm, MULT, ADD)
                            else:
                                nc.vector.tensor_scalar(
                                    ets[:].bitcast(I16), etps[:],
                                    SCHR_A, SCHR_B, MULT, ADD)
                            ets_group[tkb] = (ets, cl0)
                        for tkb in range(4 * g, 4 * g + 4):
                            ets, cl0 = ets_group.pop(tkb)
                            for hh in range(2):
                                nc.tensor.matmul(
                                    cps[hh][:, cl0:],
                                    va[p][:, tkb, 65 * hh:65 * (hh + 1)],
                                    ets[:, hh, cl0:],
                                    start=(tkb == 0),
                                    stop=(tkb == n_tkb - 1),
                                    skip_group_check=True)
                    # softmax denominators: approx reciprocal straight from
                    # PSUM (row 0 for head A / row 64 for head B), then
                    # partition-broadcast via SBUF->SBUF DMA
                    for hh in range(2):
                        zrow = wkp.tile([1, CH], F32, tag=f"zrow{hh}",
                                        name="zrow", bufs=2)
                        zrec = wkp.tile([1, CH], F32, tag=f"zrec{hh}",
                                        name="zrec", bufs=2)
                        nc.scalar.copy(zrow[:], cps[hh][64:65, :])
                        nc.vector.reciprocal_approx_fast(zrec[:], zrow[:])
                        zbh = wkp.tile([64, CH], F32, tag=f"zbh{hh}",
                                       name="zbh", bufs=2)
                        nc.gpsimd.partition_broadcast(zbh[:], zrec[:],
                                                      channels=64)
                        nc.vector.tensor_mul(
                            ctxT[p][64 * hh:64 * (hh + 1), csl],
                            cps[hh][0:64, :], zbh[:])

            def emit_outproj(c):
                # output projection for this chunk's 4 token blocks
                for j in range(4):
                    tb = 4 * c + j
                    for nb in range(2):
                        ops = ppp.tile([128, CH], F32, tag="pp", name="pp")
                        for p in range(2):
                            nc.tensor.matmul(
                                ops[:],
                                ctxT[p][:, tb * 128:(tb + 1) * 128],
                                wo_sb[:, p, nb * CH:(nb + 1) * CH],
                                start=(p == 0), stop=(p == 1))
                        osb = wkp.tile([128, CH], BF16, tag="osb", name="osb",
                                       bufs=3)
                        if nb == 0:
                            nc.scalar.copy(osb[:], ops[:])
                        else:
                            nc.vector.tensor_copy(osb[:], ops[:])
                        nc.scalar.dma_start(
                            part[tb * 128:(tb + 1) * 128,
                                 nb * CH:(nb + 1) * CH], osb[:])

            emit_proj(0)
            nc.sync.dma_start(wo_sb[:], wo[:])
            for c in range(N_CH):
                emit_attn(c)
                if c + 1 < N_CH:
                    emit_proj(c + 1)
                emit_outproj(c)
    nc.compile()
    return nc


def _get_nc():
    global _NC_CACHE
    if _NC_CACHE is None:
        _NC_CACHE = build_nc()
    return _NC_CACHE


def _pack_x(xb):
    # [S, D_MODEL] -> [128, N_CH, 8, CH]:  out[p, c, kd, t] = x[c*CH+t, kd*128+p]
    xT = xb.T.reshape(8, 128, N_CH, CH)
    return np.ascontiguousarray(xT.transpose(1, 2, 0, 3).astype(bfloat16))


def _pack_w(w):
    # [E_rows, D_MODEL] slice transposed -> [128, 8, E]
    wT = w.T.reshape(8, 128, w.shape[0])
    return np.ascontiguousarray(wT.transpose(1, 0, 2).astype(bfloat16))


def make_in_maps(query, key, value, Wq, bq, Wk, bk, Wv, bv, Wo):
    query = np.asarray(query, dtype=np.float32)
    key = np.asarray(key, dtype=np.float32)
    value = np.asarray(value, dtype=np.float32)
    in_maps = []
    for core in range(N_CORES):
        b = core // 4
        hg = core % 4
        e0 = hg * E
        esl = slice(e0, e0 + E)
        wo_c = np.asarray(Wo, np.float32)[:, esl].T  # [E, D_MODEL]
        m = {
            "xq": _pack_x(query[b]),
            "xk": _pack_x(key[b]),
            "xv": _pack_x(value[b]),
            "wq": _pack_w(np.asarray(Wq, np.float32)[esl, :]),
            "wk": _pack_w(np.asarray(Wk, np.float32)[esl, :]),
            "wv": _pack_w(np.asarray(Wv, np.float32)[esl, :]),
            "wo": np.ascontiguousarray(
                wo_c.reshape(2, 128, D_MODEL).transpose(1, 0, 2)
                .astype(bfloat16)),
            "bq": np.ascontiguousarray(
                np.asarray(bq, np.float32)[esl].reshape(2, 128).T),
            "bk": np.ascontiguousarray(
                np.asarray(bk, np.float32)[esl].reshape(2, 128).T),
        }
        in_maps.append(m)
    return in_maps


def run(inputs, trace=False):
    nc = _get_nc()
    in_maps = make_in_maps(
        inputs["query"], inputs["key"], inputs["value"],
        inputs["Wq"], inputs["bq"], inputs["Wk"], inputs["bk"],
        inputs["Wv"], inputs["bv"], inputs["Wo"])
    res = run_bass_kernel_spmd(nc, in_maps, core_ids=list(range(N_CORES)),
                               trace=trace)
    bo = np.asarray(inputs["bo"], np.float32)
    # v-bias folded out of the kernel: (ctx + bv*z)/z = ctx/z + bv, and
    # bv @ Wo^T is a constant row added here
    bvwo = np.asarray(inputs["Wo"], np.float32) @ np.asarray(
        inputs["bv"], np.float32)
    out = np.zeros((B, S, D_MODEL), np.float32)
    for core in range(N_CORES):
        out[core // 4] += res.results[core]["part"].astype(np.float32)
    out += (bo + bvwo)[None, None, :]
    return out, res


def kernel(**inputs) -> np.ndarray:
    out, _ = run(inputs, trace=False)
    return out
